# revision 36
# baseline (speedup 1.0000x reference)
"""CRF NLL loss kernel for Trainium2 (8 NeuronCores, data-parallel over batch).

Algorithm
---------
reference loss = -(mean_b[ gold_score(b) - log_norm(b) ])

log_norm comes from the forward algorithm run in *probability space* with a
constant per-step rescale kappa (folded into the transition operand):
    alpha_t = (W_f^T alpha_{t-1}) * exp(emis_t),   W_f = E * e^-kappa
Partition function via meet-in-the-middle: a forward chain over t=0..59 (59
matmul steps) and a backward chain over t=119..60 (60 matmul steps) run as
two independent full-width [K, 256] streams per core, meeting with
z = sum_j alpha_59[j] * beta_59[j]:
    beta_t = W_b^T (exp(emis_{t+1}) * beta_{t+1}),  W_b = E^T * e^-kappa
This gives one PE matmul + one DVE multiply per step at full free-dim width
(FD=256), with the two chains ping-ponging the PE and DVE engines so the
serial scan latency is hidden.

The gold-path score (emission/transition gathers at the gold tags) is
computed on host from the int tag ids; the final mean over the batch is done
on host from the per-core z outputs.
"""

import numpy as np
import ml_dtypes

import concourse.bacc as bacc_mod
import concourse.tile as tile
from concourse import mybir
from concourse.bass_utils import run_bass_kernel_spmd

B, T, K = 2048, 120, 128
NCORES = 8
BL = B // NCORES          # 256 batches per core
TH = T // 2               # 60 timesteps per direction
# chunk sizes ramp up so the scan can start after a small first DMA+exp,
# staying small enough that exp production never lags the scan
CHUNKS = [2, 2, 2, 3, 3, 4, 4, 6, 10, 12, 12]
assert sum(CHUNKS) == TH
F32 = mybir.dt.float32
BF16 = mybir.dt.bfloat16

_CACHE = {}


def _build_bass():
    """Forward/backward meet-in-the-middle program. Inputs are
    pre-transposed emissions [K, TH, BL] per direction (backward stream
    time-reversed) and the two kappa-scaled transition operands. Output is
    z[b] = sum_j a_59[j,b] * beta_59[j,b] per batch."""
    nc = bacc_mod.Bacc()
    # forward slab t and time-reversed backward slab t interleaved so one
    # DMA/exp chunk feeds both chains in lockstep
    emisI = nc.declare_dram_parameter("emisI", [K, TH, 2, BL], BF16,
                                      isOutput=False)
    wf = nc.declare_dram_parameter("wf", [K, K], BF16, isOutput=False)
    wb = nc.declare_dram_parameter("wb", [K, K], BF16, isOutput=False)
    zsum = nc.declare_dram_parameter("zsum", [1, BL], F32, isOutput=True)

    with tile.TileContext(nc) as tc:
        with (
            tc.tile_pool(name="sb", bufs=1) as sbp,
            tc.tile_pool(name="ps", bufs=2, space="PSUM") as psp,
        ):
            # all SBUF tiles share one pool (slots keyed by tag), all PSUM
            # tiles another — fewer pools means fewer teardown barriers in
            # the BSP epilogue.
            singles = chF = chB = eeFp = eeBp = stF = stB = outp = sbp
            psF = psB = psz = psp
            wf_sb = singles.tile([K, K], BF16, tag="wf", bufs=1)
            nc.gpsimd.dma_start(out=wf_sb, in_=wf[:, :])
            wb_sb = singles.tile([K, K], BF16, tag="wb", bufs=1)
            nc.gpsimd.dma_start(out=wb_sb, in_=wb[:, :])
            ones_sb = singles.tile([K, 1], BF16, tag="ones", bufs=1)
            nc.vector.memset(ones_sb, 1.0)

            # DMA + exp pipeline; ee tiles hold exp(emis) bf16. Chunk tiles
            # are allocated at the max chunk size; small leading chunks use
            # a prefix slice.
            TCMAX = max(CHUNKS)
            eeF = {}
            eeB = {}
            t0 = 0
            for ci, tn in enumerate(CHUNKS):
                ch = chF.tile([K, TCMAX, 2, BL], BF16, tag="ch", bufs=3)
                nc.sync.dma_start(out=ch[:, :tn, :, :],
                                  in_=emisI[:, t0:t0 + tn, :, :])
                ee = eeFp.tile([K, TCMAX, 2, BL], BF16, tag="ee", bufs=5)
                nc.scalar.activation(
                    out=ee[:, :tn, :, :], in_=ch[:, :tn, :, :],
                    func=mybir.ActivationFunctionType.Exp,
                )
                for ti in range(tn):
                    eeF[t0 + ti] = ee[:, ti, 0, :]
                    eeB[t0 + ti] = ee[:, ti, 1, :]
                t0 += tn

            # main interleaved scan
            # fw: a_0 = eeF[0]; step s=1..59: a_s = (wf^T a_{s-1}) * eeF[s]
            # bw: bb_0 = eeB[0]; step s=0..59: beta = wb^T bb_s;
            #     s<59: bb_{s+1} = beta * eeB[s+1];  s=59: meet
            a_sb = eeF[0]
            bb_sb = eeB[0]
            b_ps = None
            for s in range(1, TH):
                s_ps = psF.tile([K, BL], F32, tag="sf", bufs=3)
                nc.tensor.matmul(s_ps, lhsT=wf_sb, rhs=a_sb,
                                 start=True, stop=True)
                a_new = stF.tile([K, BL], BF16, tag="af", bufs=4)
                nc.vector.tensor_mul(a_new, s_ps, eeF[s])
                a_sb = a_new

                b_ps = psB.tile([K, BL], F32, tag="sb", bufs=3)
                nc.tensor.matmul(b_ps, lhsT=wb_sb, rhs=bb_sb,
                                 start=True, stop=True)
                bb_new = stB.tile([K, BL], BF16, tag="bf", bufs=4)
                nc.vector.tensor_mul(bb_new, b_ps, eeB[s])
                bb_sb = bb_new

            # bw has one more matmul than the fw loop (60 vs 59)
            b_ps = psB.tile([K, BL], F32, tag="sb", bufs=3)
            nc.tensor.matmul(b_ps, lhsT=wb_sb, rhs=bb_sb,
                             start=True, stop=True)

            # meet: ab = a_59 * beta_59  (beta in PSUM), then z = ones^T ab
            ab_sb = outp.tile([K, BL], BF16, tag="ab", bufs=1)
            nc.vector.tensor_mul(ab_sb, b_ps, a_sb)
            z_ps = psz.tile([1, BL], F32, tag="z", bufs=1)
            nc.tensor.matmul(z_ps, lhsT=ones_sb, rhs=ab_sb,
                             start=True, stop=True)
            z_sb = outp.tile([1, BL], F32, tag="zsb", bufs=1)
            nc.scalar.copy(out=z_sb, in_=z_ps)
            nc.sync.dma_start(out=zsum[:, :], in_=z_sb)
    nc.finalize()
    return nc


def _kappa(trans):
    E = np.exp(trans)
    return float(np.log(E.sum(0).mean()) + 0.5)


def _make_in_maps(emissions, transitions):
    em = np.ascontiguousarray(emissions, dtype=np.float32)
    trans = np.ascontiguousarray(transitions, dtype=np.float32)
    E = np.exp(trans)                                   # [K, K]
    kappa = _kappa(trans)
    wf = (E * np.exp(-kappa)).astype(ml_dtypes.bfloat16)
    wb = (E.T * np.exp(-kappa)).astype(ml_dtypes.bfloat16)
    in_maps = []
    for c in range(NCORES):
        shard = em[c * BL:(c + 1) * BL]                 # [BL, T, K]
        emisT = shard.transpose(2, 1, 0).astype(ml_dtypes.bfloat16)  # [K,T,BL]
        emI = np.ascontiguousarray(
            np.stack([emisT[:, :TH, :], emisT[:, :TH - 1:-1, :]], axis=2))
        in_maps.append({"emisI": emI, "wf": wf, "wb": wb})
    return in_maps


def _numpy_fallback(emissions, tag_ids, mask, transitions):
    """Exact reference math in numpy; used only for inputs the specialized
    device program does not cover (general mask / other shapes)."""
    em = np.asarray(emissions, np.float64)
    maskf = np.asarray(mask, np.float64)
    tl = np.asarray(tag_ids).astype(np.int64)
    tr = np.asarray(transitions, np.float64)
    unary = np.take_along_axis(em, tl[..., None], axis=2)[..., 0]
    score = (unary * maskf).sum(1) + \
        (tr[tl[:, :-1], tl[:, 1:]] * maskf[:, :-1] * maskf[:, 1:]).sum(1)
    alpha = em[:, 0, :]
    for t in range(1, em.shape[1]):
        m = alpha.max(1, keepdims=True)
        new = np.log(np.exp(alpha - m) @ np.exp(tr)) + m + em[:, t, :]
        alpha = np.where(maskf[:, t:t + 1] > 0, new, alpha)
    m = alpha.max(1, keepdims=True)
    logz = np.log(np.exp(alpha - m).sum(1)) + m[:, 0]
    return np.float32(-(score - logz).mean())


def kernel(emissions, tag_ids, mask, transitions):
    em = np.ascontiguousarray(emissions, dtype=np.float32)
    tags = np.asarray(tag_ids)
    trans = np.ascontiguousarray(transitions, dtype=np.float32)

    if em.shape != (B, T, K) or not np.all(np.asarray(mask) == 1):
        return _numpy_fallback(emissions, tag_ids, mask, transitions)

    kappa = _kappa(trans)

    if "nc" not in _CACHE:
        _CACHE["nc"] = _build_bass()
    nc = _CACHE["nc"]

    in_maps = _make_in_maps(em, trans)

    res = run_bass_kernel_spmd(nc, in_maps, core_ids=list(range(NCORES)))

    # gold-path score (gather at gold tags) + final reduction
    tl = tags.astype(np.int64)
    unary = np.take_along_axis(em, tl[..., None], axis=2)[..., 0].sum(1)
    binary = trans[tl[:, :-1], tl[:, 1:]].sum(1)
    score = unary + binary                              # [B]

    logz = np.empty(B, np.float32)
    for c in range(NCORES):
        z = res.results[c]["zsum"][0]                   # [BL]
        logz[c * BL:(c + 1) * BL] = np.log(z) + (T - 1) * kappa

    loss = -(score.astype(np.float64) - logz.astype(np.float64)).mean()
    return np.float32(loss)


# revision 38
# speedup vs baseline: 1.0222x; 1.0222x over previous
"""CRF NLL loss kernel for Trainium2 (8 NeuronCores, data-parallel over batch).

Algorithm
---------
reference loss = -(mean_b[ gold_score(b) - log_norm(b) ])

log_norm comes from the forward algorithm run in *probability space* with a
constant per-step rescale kappa (folded into the transition operand):
    alpha_t = (W_f^T alpha_{t-1}) * exp(emis_t),   W_f = E * e^-kappa
Partition function via meet-in-the-middle: a forward chain over t=0..59 (59
matmul steps) and a backward chain over t=119..60 (60 matmul steps) run as
two independent full-width [K, 256] streams per core, meeting with
z = sum_j alpha_59[j] * beta_59[j]:
    beta_t = W_b^T (exp(emis_{t+1}) * beta_{t+1}),  W_b = E^T * e^-kappa
This gives one PE matmul + one DVE multiply per step at full free-dim width
(FD=256), with the two chains ping-ponging the PE and DVE engines so the
serial scan latency is hidden.

The gold-path score (emission/transition gathers at the gold tags) is
computed on host from the int tag ids; the final mean over the batch is done
on host from the per-core z outputs.
"""

import numpy as np
import ml_dtypes

import concourse.bacc as bacc_mod
import concourse.tile as tile
from concourse import mybir
from concourse.bass_utils import run_bass_kernel_spmd

B, T, K = 2048, 120, 128
NCORES = 8
BL = B // NCORES          # 256 batches per core
TH = T // 2               # 60 timesteps per direction
# chunk sizes ramp up so the scan can start after a small first DMA+exp,
# staying small enough that exp production never lags the scan
CHUNKS = [2, 2, 2, 3, 3, 4, 4, 6, 10, 12, 12]
assert sum(CHUNKS) == TH
F32 = mybir.dt.float32
BF16 = mybir.dt.bfloat16

_CACHE = {}


def _build_bass():
    """Forward/backward meet-in-the-middle program. Inputs are
    pre-transposed emissions [K, TH, BL] per direction (backward stream
    time-reversed) and the two kappa-scaled transition operands. Output is
    z[b] = sum_j a_59[j,b] * beta_59[j,b] per batch."""
    nc = bacc_mod.Bacc()
    # forward slab t and time-reversed backward slab t interleaved so one
    # DMA/exp chunk feeds both chains in lockstep
    emisI = nc.declare_dram_parameter("emisI", [K, TH, 2, BL], BF16,
                                      isOutput=False)
    wf = nc.declare_dram_parameter("wf", [K, K], BF16, isOutput=False)
    wb = nc.declare_dram_parameter("wb", [K, K], BF16, isOutput=False)
    zsum = nc.declare_dram_parameter("zsum", [1, BL], F32, isOutput=True)

    with tile.TileContext(nc) as tc:
        with (
            tc.tile_pool(name="sb", bufs=1) as sbp,
            tc.tile_pool(name="ps", bufs=2, space="PSUM") as psp,
        ):
            # all SBUF tiles share one pool (slots keyed by tag), all PSUM
            # tiles another — fewer pools means fewer teardown barriers in
            # the BSP epilogue.
            singles = chF = chB = eeFp = eeBp = stF = stB = outp = sbp
            psF = psB = psz = psp
            wf_sb = singles.tile([K, K], BF16, tag="wf", bufs=1)
            nc.gpsimd.dma_start(out=wf_sb, in_=wf[:, :])
            wb_sb = singles.tile([K, K], BF16, tag="wb", bufs=1)
            nc.gpsimd.dma_start(out=wb_sb, in_=wb[:, :])
            ones_sb = singles.tile([K, 1], BF16, tag="ones", bufs=1)
            nc.vector.memset(ones_sb, 1.0)

            # DMA + exp pipeline; ee tiles hold exp(emis) bf16. Chunk tiles
            # are allocated at the max chunk size; small leading chunks use
            # a prefix slice.
            TCMAX = max(CHUNKS)
            eeF = {}
            eeB = {}
            t0 = 0
            for ci, tn in enumerate(CHUNKS):
                ch = chF.tile([K, TCMAX, 2, BL], BF16, tag="ch", bufs=3)
                nc.sync.dma_start(out=ch[:, :tn, :, :],
                                  in_=emisI[:, t0:t0 + tn, :, :])
                ee = eeFp.tile([K, TCMAX, 2, BL], BF16, tag="ee", bufs=5)
                nc.scalar.activation(
                    out=ee[:, :tn, :, :], in_=ch[:, :tn, :, :],
                    func=mybir.ActivationFunctionType.Exp,
                )
                for ti in range(tn):
                    eeF[t0 + ti] = ee[:, ti, 0, :]
                    eeB[t0 + ti] = ee[:, ti, 1, :]
                t0 += tn

            # main interleaved scan
            # fw: a_0 = eeF[0]; step s=1..59: a_s = (wf^T a_{s-1}) * eeF[s]
            # bw: bb_0 = eeB[0]; step s=0..59: beta = wb^T bb_s;
            #     s<59: bb_{s+1} = beta * eeB[s+1];  s=59: meet
            a_sb = eeF[0]
            bb_sb = eeB[0]
            b_ps = None
            for s in range(1, TH):
                s_ps = psF.tile([K, BL], F32, tag="sf", bufs=3)
                nc.tensor.matmul(s_ps, lhsT=wf_sb, rhs=a_sb,
                                 start=True, stop=True)
                a_new = stF.tile([K, BL], BF16, tag="af", bufs=4)
                nc.vector.scalar_tensor_tensor(
                    a_new, s_ps, 1.0, eeF[s],
                    mybir.AluOpType.mult, mybir.AluOpType.mult)
                a_sb = a_new

                b_ps = psB.tile([K, BL], F32, tag="sb", bufs=3)
                nc.tensor.matmul(b_ps, lhsT=wb_sb, rhs=bb_sb,
                                 start=True, stop=True)
                bb_new = stB.tile([K, BL], BF16, tag="bf", bufs=4)
                nc.vector.scalar_tensor_tensor(
                    bb_new, b_ps, 1.0, eeB[s],
                    mybir.AluOpType.mult, mybir.AluOpType.mult)
                bb_sb = bb_new

            # bw has one more matmul than the fw loop (60 vs 59)
            b_ps = psB.tile([K, BL], F32, tag="sb", bufs=3)
            nc.tensor.matmul(b_ps, lhsT=wb_sb, rhs=bb_sb,
                             start=True, stop=True)

            # meet: ab = a_59 * beta_59  (beta in PSUM), then z = ones^T ab
            ab_sb = outp.tile([K, BL], BF16, tag="ab", bufs=1)
            nc.vector.tensor_mul(ab_sb, b_ps, a_sb)
            z_ps = psz.tile([1, BL], F32, tag="z", bufs=1)
            nc.tensor.matmul(z_ps, lhsT=ones_sb, rhs=ab_sb,
                             start=True, stop=True)
            z_sb = outp.tile([1, BL], F32, tag="zsb", bufs=1)
            nc.scalar.copy(out=z_sb, in_=z_ps)
            nc.sync.dma_start(out=zsum[:, :], in_=z_sb)
    nc.finalize()
    return nc


def _kappa(trans):
    E = np.exp(trans)
    return float(np.log(E.sum(0).mean()) + 0.5)


def _make_in_maps(emissions, transitions):
    em = np.ascontiguousarray(emissions, dtype=np.float32)
    trans = np.ascontiguousarray(transitions, dtype=np.float32)
    E = np.exp(trans)                                   # [K, K]
    kappa = _kappa(trans)
    wf = (E * np.exp(-kappa)).astype(ml_dtypes.bfloat16)
    wb = (E.T * np.exp(-kappa)).astype(ml_dtypes.bfloat16)
    in_maps = []
    for c in range(NCORES):
        shard = em[c * BL:(c + 1) * BL]                 # [BL, T, K]
        emisT = shard.transpose(2, 1, 0).astype(ml_dtypes.bfloat16)  # [K,T,BL]
        emI = np.ascontiguousarray(
            np.stack([emisT[:, :TH, :], emisT[:, :TH - 1:-1, :]], axis=2))
        in_maps.append({"emisI": emI, "wf": wf, "wb": wb})
    return in_maps


def _numpy_fallback(emissions, tag_ids, mask, transitions):
    """Exact reference math in numpy; used only for inputs the specialized
    device program does not cover (general mask / other shapes)."""
    em = np.asarray(emissions, np.float64)
    maskf = np.asarray(mask, np.float64)
    tl = np.asarray(tag_ids).astype(np.int64)
    tr = np.asarray(transitions, np.float64)
    unary = np.take_along_axis(em, tl[..., None], axis=2)[..., 0]
    score = (unary * maskf).sum(1) + \
        (tr[tl[:, :-1], tl[:, 1:]] * maskf[:, :-1] * maskf[:, 1:]).sum(1)
    alpha = em[:, 0, :]
    for t in range(1, em.shape[1]):
        m = alpha.max(1, keepdims=True)
        new = np.log(np.exp(alpha - m) @ np.exp(tr)) + m + em[:, t, :]
        alpha = np.where(maskf[:, t:t + 1] > 0, new, alpha)
    m = alpha.max(1, keepdims=True)
    logz = np.log(np.exp(alpha - m).sum(1)) + m[:, 0]
    return np.float32(-(score - logz).mean())


def kernel(emissions, tag_ids, mask, transitions):
    em = np.ascontiguousarray(emissions, dtype=np.float32)
    tags = np.asarray(tag_ids)
    trans = np.ascontiguousarray(transitions, dtype=np.float32)

    if em.shape != (B, T, K) or not np.all(np.asarray(mask) == 1):
        return _numpy_fallback(emissions, tag_ids, mask, transitions)

    kappa = _kappa(trans)

    if "nc" not in _CACHE:
        _CACHE["nc"] = _build_bass()
    nc = _CACHE["nc"]

    in_maps = _make_in_maps(em, trans)

    res = run_bass_kernel_spmd(nc, in_maps, core_ids=list(range(NCORES)))

    # gold-path score (gather at gold tags) + final reduction
    tl = tags.astype(np.int64)
    unary = np.take_along_axis(em, tl[..., None], axis=2)[..., 0].sum(1)
    binary = trans[tl[:, :-1], tl[:, 1:]].sum(1)
    score = unary + binary                              # [B]

    logz = np.empty(B, np.float32)
    for c in range(NCORES):
        z = res.results[c]["zsum"][0]                   # [BL]
        logz[c * BL:(c + 1) * BL] = np.log(z) + (T - 1) * kappa

    loss = -(score.astype(np.float64) - logz.astype(np.float64)).mean()
    return np.float32(loss)
